# revision 15
# baseline (speedup 1.0000x reference)
"""Trainium2 Bass kernel for nn_BinaryBiaffine2 (biaffine dependency scorer).

Math (per batch b):
    h_dep  = leaky_relu(hidden @ W_dep  + b_dep)             [L, 500]
    h_head = leaky_relu(hidden @ W_head + b_head)            [L, 500]
    dep    = h_dep  @ Wc[:500]                               [L, 2]
    head   = h_head @ Wc[500:]                               [L, 2]
    out[i, j, c] = dep[i, c] + head[j, c] + bc[c]            [L, L, 2]

Sharding: data-parallel over batch, 2 batches per core on 8 cores.

v3 strategy (vs v2's 108.5us):
  - hidden is transposed to [D, L] on the HOST and fed as bf16, so the
    kernel streams hT tiles [d=128, tok] straight from DRAM: no PE
    transposes, no PSUM round-trip, no DVE copies for them.
  - weights bf16 (1 cycle/row on PE, half the DMA bytes).
  - leaky_relu fused into ONE scalar-engine activation (Lrelu, alpha).
  - head scores [2, L] via M=2 matmuls; partition-broadcast via a
    ones-row matmul (f32r); +bc folded into the PSUM->SBUF copy.
  - dep scores via per-i-tile tiny matmuls: out[i(128-part), 2] =
    lhsT(lh_dep[:, chunk]) @ wc_dep, accumulated over m-tiles; ap=2 so
    they are nearly free on the PE.
  - out store in bf16 (host upcasts to f32): halves the 16.8MB/core
    output DMA.  rel-err budget 2e-2 >> bf16 rounding ~2e-3.
  - PE-stall-aware emission: dependent PE groups are emitted >=1 mlp
    group after their producers; dummy warmup matmuls keep the PE busy
    (and its p-state ramped) while batch-0 hidden streams in.
"""

import os
import sys

if "/opt/trn_rl_repo" not in sys.path:
    sys.path.insert(0, "/opt/trn_rl_repo")

import numpy as np

B, L, D = 16, 1024, 1024
MLP = 500
MLP_PAD = 512
NEG_SLOPE = 0.01
N_CORES = 8
B_PER_CORE = B // N_CORES
P = 128
N_MT = MLP_PAD // P  # 4 m-tiles of 128
N_KO = D // P        # 8 d-slices of 128
N_TSUB = L // P      # 8 token subtiles per batch

WARMUP = int(os.environ.get("BB_WARMUP", "14"))

_CACHE = {}


def _build_nc():
    import concourse.tile as tile
    from concourse import bacc, mybir
    from concourse.bass import ts
    from contextlib import ExitStack

    f32 = mybir.dt.float32
    f32r = mybir.dt.float32r
    bf16 = mybir.dt.bfloat16
    Lrelu = mybir.ActivationFunctionType.Lrelu
    Identity = mybir.ActivationFunctionType.Identity
    Add = mybir.AluOpType.add

    nc = bacc.Bacc()

    hid_d = nc.dram_tensor("hidden_t", [B_PER_CORE, D, L], bf16, kind="ExternalInput")
    w_dep_d = nc.dram_tensor("w_dep", [D, MLP_PAD], bf16, kind="ExternalInput")
    w_head_d = nc.dram_tensor("w_head", [D, MLP_PAD], bf16, kind="ExternalInput")
    # f32 consts: cols 0-3 = b_dep per m-tile, 4-7 = b_head, 8-9 = bc
    cf32_d = nc.dram_tensor("consts_f32", [P, 2 * N_MT + 2], f32, kind="ExternalInput")
    wc_dep_d = nc.dram_tensor("wc_dep_t", [P, N_MT, 2], bf16, kind="ExternalInput")
    wc_head_d = nc.dram_tensor("wc_head_t", [P, N_MT, 33], bf16, kind="ExternalInput")
    ones_d = nc.dram_tensor("ones_r", [33, P], f32r, kind="ExternalInput")
    out_d = nc.dram_tensor("out", [B_PER_CORE, L, L, 2], bf16, kind="ExternalOutput")

    with tile.TileContext(nc) as tc:
        with ExitStack() as ctx:
            const = ctx.enter_context(tc.tile_pool(name="const", bufs=1))
            hT_p = ctx.enter_context(tc.tile_pool(name="hT", bufs=2 * N_KO))
            lhh_p = ctx.enter_context(tc.tile_pool(name="lhh", bufs=N_MT))
            lhd_p = ctx.enter_context(tc.tile_pool(name="lhd", bufs=2 * N_MT))
            hs_p = ctx.enter_context(tc.tile_pool(name="hs", bufs=2))
            hbc_p = ctx.enter_context(tc.tile_pool(name="hbc", bufs=4))
            dsb_p = ctx.enter_context(tc.tile_pool(name="dsb", bufs=4))
            out_p = ctx.enter_context(tc.tile_pool(name="outp", bufs=6))
            mlp_ps = ctx.enter_context(tc.tile_pool(name="mlpps", bufs=2, space="PSUM"))
            sc_ps = ctx.enter_context(tc.tile_pool(name="scps", bufs=2, space="PSUM"))

            # ---- constant / weight loads -------------------------------
            # sync: ones (warmup input) first, then batch-0 hidden even kos,
            # then f32 consts + wc tiles.  scalar: w_head chunk for ko0-3,
            # batch-0 hidden odd kos, w_head ko4-7.  gpsimd: w_dep, b1 hidden.
            warm_in = const.tile([1, 512], bf16)
            nc.vector.memset(warm_in, 0.0)
            # trigger both ACT table narrowings during startup idle so no
            # mid-kernel InstLoadActFuncSet lands on the critical path
            warm_act = const.tile([1, 8], f32)
            nc.scalar.activation(warm_act, warm_in[:, 0:8], Lrelu, alpha=NEG_SLOPE)
            nc.scalar.activation(warm_act, warm_in[:, 0:8], Identity)
            w_sb = {}
            w_head_sb = const.tile([P, N_KO, MLP_PAD], bf16)
            w_dep_sb = const.tile([P, N_KO, MLP_PAD], bf16)
            w_sb["dep"], w_sb["head"] = w_dep_sb, w_head_sb

            # hidden tiles: ko-pairs hTp[b][pi] = [P, 2, L] bf16
            hTp = [[hT_p.tile([P, 2, L], bf16, name="hT") for _ in range(N_KO // 2)]
                   for _ in range(B_PER_CORE)]

            def hT(b, ko):
                return hTp[b][ko // 2][:, ko % 2]

            # startup order tuned for batch-0 mt0 pacing: sync carries pairs
            # (0,1),(4,5); scalar interleaves w_head ko-chunks with pairs
            nc.scalar.dma_start(
                w_head_sb[:, 0:2, :],
                w_head_d[0 : 2 * P, :].rearrange("(k p) m -> p k m", p=P),
            )
            nc.sync.dma_start(
                hTp[0][0], hid_d[0, ts(0, 2 * P), :].rearrange("(k p) l -> p k l", p=P)
            )
            nc.scalar.dma_start(
                hTp[0][1], hid_d[0, ts(1, 2 * P), :].rearrange("(k p) l -> p k l", p=P)
            )
            nc.sync.dma_start(
                hTp[0][2], hid_d[0, ts(2, 2 * P), :].rearrange("(k p) l -> p k l", p=P)
            )
            nc.scalar.dma_start(
                w_head_sb[:, 2:4, :],
                w_head_d[2 * P : 4 * P, :].rearrange("(k p) m -> p k m", p=P),
            )
            nc.sync.dma_start(
                hTp[0][3], hid_d[0, ts(3, 2 * P), :].rearrange("(k p) l -> p k l", p=P)
            )
            nc.scalar.dma_start(
                w_head_sb[:, 4:8, :],
                w_head_d[4 * P : 8 * P, :].rearrange("(k p) m -> p k m", p=P),
            )

            ones_sb = const.tile([33, P], f32r)
            nc.sync.dma_start(ones_sb, ones_d[:, :])
            ones_row = {0: ones_sb[0:1, :], 1: ones_sb[32:33, :]}
            cf32_sb = const.tile([P, 2 * N_MT + 2], f32)
            nc.sync.dma_start(cf32_sb, cf32_d[:, :])
            b_sb = {"dep": cf32_sb[:, 0:N_MT], "head": cf32_sb[:, N_MT : 2 * N_MT]}
            bc_sb = cf32_sb[:, 2 * N_MT : 2 * N_MT + 2]
            wc_dep_sb = const.tile([P, N_MT, 2], bf16)
            nc.sync.dma_start(wc_dep_sb, wc_dep_d[:, :, :])
            wc_head_sb = const.tile([P, N_MT, 33], bf16)
            nc.sync.dma_start(wc_head_sb, wc_head_d[:, :, :])

            nc.gpsimd.dma_start(
                w_dep_sb,
                w_dep_d[:, :].rearrange("(k p) m -> p k m", p=P),
            )
            for pi in range(4):
                nc.gpsimd.dma_start(
                    hTp[1][pi],
                    hid_d[1, ts(pi, 2 * P), :].rearrange("(k p) l -> p k l", p=P),
                )

            # ---- emission helpers --------------------------------------
            def emit_dummy(n, ap=512):
                # keep the PE busy/p-state-warm; bf16 => 1 cycle/row
                for _ in range(n):
                    wps = sc_ps.tile([P, ap], f32, name="sc", padded_shape=[P, 1024])
                    nc.tensor.matmul(wps, lhsT=warm_in[:, 0:P], rhs=warm_in[:, 0:ap],
                                     start=True, stop=True)

            lh_head = {}   # (b, mt) -> [P, L] bf16
            lh_dep = {}    # (b, half, mt) -> [P, 512] bf16

            def emit_head_mlp(b, mt, pace=False):
                ps = mlp_ps.tile([P, 1024], f32, name="mlp")
                for ko in range(N_KO):
                    for half in range(2):
                        nc.tensor.matmul(
                            ps[:, ts(half, 512)],
                            lhsT=w_sb["head"][:, ko, ts(mt, P)],
                            rhs=hT(b, ko)[:, ts(half, 512)],
                            start=(ko == 0),
                            stop=(ko == N_KO - 1),
                        )
                    if pace and ko < N_KO - 1:
                        emit_dummy(1, ap=256)
                lh = lhh_p.tile([P, L], bf16, name="lh")
                nc.scalar.activation(lh, ps, Lrelu, bias=b_sb["head"][:, mt : mt + 1],
                                     alpha=NEG_SLOPE)
                lh_head[b, mt] = lh

            def emit_dep_mlp(b, quarter, mt):
                ps = mlp_ps.tile([P, 256], f32, name="mlp", padded_shape=[P, 1024])
                for ko in range(N_KO):
                    nc.tensor.matmul(
                        ps,
                        lhsT=w_sb["dep"][:, ko, ts(mt, P)],
                        rhs=hT(b, ko)[:, ts(quarter, 256)],
                        start=(ko == 0),
                        stop=(ko == N_KO - 1),
                    )
                lh = lhd_p.tile([P, 256], bf16, name="lhd")
                nc.scalar.activation(lh, ps, Lrelu, bias=b_sb["dep"][:, mt : mt + 1],
                                     alpha=NEG_SLOPE)
                lh_dep[b, quarter, mt] = lh

            hs_ps_t = {}
            hs_sb_t = {}

            def emit_hs(b, mt):
                # head scores [2, L]: accumulate over m-tiles, per 512-half
                if mt == 0:
                    hs_ps_t[b] = sc_ps.tile([33, L], f32, name="sc",
                                            padded_shape=[P, 1024])
                for half in range(2):
                    nc.tensor.matmul(
                        hs_ps_t[b][:, ts(half, 512)],
                        lhsT=wc_head_sb[:, mt, :],
                        rhs=lh_head[b, mt][:, ts(half, 512)],
                        start=(mt == 0),
                        stop=(mt == N_MT - 1),
                    )
                if mt == N_MT - 1:
                    hs = hs_p.tile([33, L], f32r, name="hs_sb")
                    nc.vector.tensor_copy(hs, hs_ps_t[b])
                    hs_sb_t[b] = hs

            head_bc = {}

            def emit_bc(b, c):
                ps = sc_ps.tile([P, 1024], f32, name="sc")
                for half in range(2):
                    nc.tensor.matmul(
                        ps[:, ts(half, 512)],
                        lhsT=ones_row[c],
                        rhs=hs_sb_t[b][32 * c : 32 * c + 1, ts(half, 512)],
                        start=True,
                        stop=True,
                    )
                hb = hbc_p.tile([P, L], f32, name="hb")
                nc.vector.tensor_scalar(hb, ps, bc_sb[:, c : c + 1], None, Add)
                head_bc[b, c] = hb

            def emit_tiny_and_out(b, quarter, last=False):
                # dep scores for this quarter: [P(tok), 2] per i-tile
                tiny = sc_ps.tile([P, 2 * 2], f32, name="sc",
                                  padded_shape=[P, 1024])
                for qq in range(2):
                    for mt in range(N_MT):
                        nc.tensor.matmul(
                            tiny[:, 2 * qq : 2 * qq + 2],
                            lhsT=lh_dep[b, quarter, mt][:, ts(qq, P)],
                            rhs=wc_dep_sb[:, mt, :],
                            start=(mt == 0),
                            stop=(mt == N_MT - 1),
                        )
                dsb = dsb_p.tile([P, 2 * 2], f32, name="dsb")
                nc.vector.tensor_copy(dsb, tiny)
                # pairwise add + store.  ACT stays mostly free for lrelu
                # evacuations; paired DMAs on SP/Pool, last quarter split
                # across SP + scalar for the shortest tail.
                def op(sel, dst, srch, dap):
                    if sel == 0:
                        nc.vector.tensor_scalar(dst, srch, dap, None, Add)
                    elif sel == 1:
                        nc.scalar.activation(dst, srch, Identity, bias=dap)
                    else:
                        nc.gpsimd.tensor_scalar(dst, srch, dap, None, Add)

                last_batch = b == B_PER_CORE - 1
                if last_batch and quarter >= 2:
                    # endgame: singles only; no ACT out-ops for Q2 (its SEQ
                    # must stay clear for Q3's lrelus), ACT allowed for Q3's
                    # first tile only (after the final lrelu).  Queues: Q2 on
                    # sync (transfers drain before Q3's), Q3 on scalar+sync.
                    if quarter == 2:
                        units = [((0, 2), nc.sync), ((2, 0), nc.sync)]
                    else:
                        units = [((0, 1), nc.scalar), ((2, 0), nc.sync)]
                    for s, (pk, eng) in enumerate(units):
                        tsub = 2 * quarter + s
                        ot = out_p.tile([P, L, 2], bf16, name="otl")
                        d0 = dsb[:, 2 * s : 2 * s + 1]
                        d1 = dsb[:, 2 * s + 1 : 2 * s + 2]
                        op(pk[0], ot[:, :, 0], head_bc[b, 0], d0)
                        op(pk[1], ot[:, :, 1], head_bc[b, 1], d1)
                        eng.dma_start(out_d[b, ts(tsub, P)], ot)
                    return
                picks = [(0, 2), (1, 0)] if quarter % 2 == 0 else [(0, 1), (2, 0)]
                ot = out_p.tile([P, 2, L, 2], bf16, name="ot")
                for s in range(2):
                    d0 = dsb[:, 2 * s : 2 * s + 1]
                    d1 = dsb[:, 2 * s + 1 : 2 * s + 2]
                    op(picks[s][0], ot[:, s, :, 0], head_bc[b, 0], d0)
                    op(picks[s][1], ot[:, s, :, 1], head_bc[b, 1], d1)
                eng = nc.sync if quarter % 2 == 0 else nc.gpsimd
                eng.dma_start(
                    out_d[b, ts(quarter, 2 * P)].rearrange(
                        "(s p) j c -> p s j c", p=P
                    ),
                    ot,
                )

            # ---- schedule ----------------------------------------------
            # Interleaving keeps every dependent PE group >=1 mlp group
            # behind its producer so the PE never stalls.
            for b in range(B_PER_CORE):
                if b == 0:
                    emit_dummy(WARMUP)
                    emit_head_mlp(b, 0, pace=True)
                    emit_head_mlp(b, 1)
                # for b>0, head mlp 0/1 were emitted inside batch b-1
                if b == 0:
                    # mt0/mt1 are DMA-paced at startup: keep hs well behind
                    emit_head_mlp(b, 2)
                    emit_hs(b, 0)
                    emit_head_mlp(b, 3)
                    emit_hs(b, 1)
                    emit_dep_mlp(b, 0, 0)
                    emit_hs(b, 2)
                else:
                    emit_hs(b, 0)
                    emit_head_mlp(b, 2)
                    emit_hs(b, 1)
                    emit_head_mlp(b, 3)
                    emit_hs(b, 2)
                    emit_dep_mlp(b, 0, 0)
                emit_dep_mlp(b, 0, 1)
                emit_dep_mlp(b, 0, 2)
                emit_dep_mlp(b, 0, 3)
                emit_hs(b, 3)
                emit_dep_mlp(b, 1, 0)
                emit_dep_mlp(b, 1, 1)
                emit_bc(b, 0)
                emit_dep_mlp(b, 1, 2)
                emit_dep_mlp(b, 1, 3)
                emit_bc(b, 1)
                emit_tiny_and_out(b, 0)
                emit_dep_mlp(b, 2, 0)
                emit_dep_mlp(b, 2, 1)
                emit_dep_mlp(b, 2, 2)
                emit_dep_mlp(b, 2, 3)
                emit_tiny_and_out(b, 1)
                emit_dep_mlp(b, 3, 0)
                emit_dep_mlp(b, 3, 1)
                if b + 1 < B_PER_CORE:
                    emit_dep_mlp(b, 3, 2)
                    emit_dep_mlp(b, 3, 3)
                    emit_tiny_and_out(b, 2)
                    emit_head_mlp(b + 1, 0)
                    emit_tiny_and_out(b, 3)
                    emit_head_mlp(b + 1, 1)
                else:
                    # last batch: drain quarter 2 before Q3's mlp finishes so
                    # only Q3's chain remains in the tail
                    emit_tiny_and_out(b, 2)
                    emit_dep_mlp(b, 3, 2)
                    emit_dep_mlp(b, 3, 3)
                    emit_tiny_and_out(b, 3, last=True)

    nc.compile()
    return nc


def _prep_consts(W_dep, b_dep, W_head, b_head, Wc, bc):
    import ml_dtypes

    f = np.float32
    bf = ml_dtypes.bfloat16

    def pad_w(W):
        Wp = np.zeros((D, MLP_PAD), f)
        Wp[:, :MLP] = W
        return Wp.astype(bf)

    def bias_t(bvec):
        bp = np.zeros((MLP_PAD,), f)
        bp[:MLP] = bvec
        return bp.reshape(N_MT, P).T  # [P, N_MT]

    cf32 = np.empty((P, 2 * N_MT + 2), f)
    cf32[:, 0:N_MT] = bias_t(b_dep)
    cf32[:, N_MT : 2 * N_MT] = bias_t(b_head)
    cf32[:, 2 * N_MT :] = np.broadcast_to(bc.astype(f), (P, 2))

    def wc_t(wc_half, width=2, stride=1):
        wcp = np.zeros((MLP_PAD, 2), f)
        wcp[:MLP] = wc_half
        wct = wcp.reshape(N_MT, P, 2).transpose(1, 0, 2)  # [P, N_MT, 2]
        out = np.zeros((P, N_MT, width), f)
        out[:, :, 0] = wct[:, :, 0]
        out[:, :, stride] = wct[:, :, 1]
        return out.astype(bf).copy()

    return {
        "w_dep": pad_w(W_dep),
        "w_head": pad_w(W_head),
        "consts_f32": cf32,
        "wc_dep_t": wc_t(Wc[:MLP]),
        "wc_head_t": wc_t(Wc[MLP:], width=33, stride=32),
        "ones_r": np.ones((33, P), f),
    }


def kernel(hidden_state, W_dep, b_dep, W_head, b_head, Wc, bc):
    import ml_dtypes
    from concourse.bass_utils import run_bass_kernel_spmd

    bf = ml_dtypes.bfloat16
    hidden_state = np.asarray(hidden_state, dtype=np.float32)
    consts = _prep_consts(
        np.asarray(W_dep, np.float32),
        np.asarray(b_dep, np.float32),
        np.asarray(W_head, np.float32),
        np.asarray(b_head, np.float32),
        np.asarray(Wc, np.float32),
        np.asarray(bc, np.float32),
    )

    if "nc" not in _CACHE:
        _CACHE["nc"] = _build_nc()
    nc = _CACHE["nc"]

    hbf = hidden_state.astype(bf)
    in_maps = []
    for k in range(N_CORES):
        sl = hbf[k * B_PER_CORE : (k + 1) * B_PER_CORE]
        m = {"hidden_t": np.ascontiguousarray(sl.transpose(0, 2, 1))}
        m.update(consts)
        in_maps.append(m)

    trace = bool(int(os.environ.get("BB_TRACE", "0")))
    if not trace:
        # The NTFF profiling hook (antenv.axon_hooks) is absent in this
        # container; a stray BASS_TRACE=1 would crash the run. Force off.
        os.environ["BASS_NEVER_TRACE"] = "1"
    res = run_bass_kernel_spmd(nc, in_maps, list(range(N_CORES)), trace=trace)
    _CACHE["last_results"] = res
    out = np.concatenate(
        [np.asarray(res.results[k]["out"], dtype=np.float32) for k in range(N_CORES)],
        axis=0,
    )
    return out


# revision 16
# speedup vs baseline: 1.0215x; 1.0215x over previous
"""Trainium2 Bass kernel for nn_BinaryBiaffine2 (biaffine dependency scorer).

Math (per batch b):
    h_dep  = leaky_relu(hidden @ W_dep  + b_dep)             [L, 500]
    h_head = leaky_relu(hidden @ W_head + b_head)            [L, 500]
    dep    = h_dep  @ Wc[:500]                               [L, 2]
    head   = h_head @ Wc[500:]                               [L, 2]
    out[i, j, c] = dep[i, c] + head[j, c] + bc[c]            [L, L, 2]

Sharding: data-parallel over batch, 2 batches per core on 8 cores.

v3 strategy (vs v2's 108.5us):
  - hidden is transposed to [D, L] on the HOST and fed as bf16, so the
    kernel streams hT tiles [d=128, tok] straight from DRAM: no PE
    transposes, no PSUM round-trip, no DVE copies for them.
  - weights bf16 (1 cycle/row on PE, half the DMA bytes).
  - leaky_relu fused into ONE scalar-engine activation (Lrelu, alpha).
  - head scores [2, L] via M=2 matmuls; partition-broadcast via a
    ones-row matmul (f32r); +bc folded into the PSUM->SBUF copy.
  - dep scores via per-i-tile tiny matmuls: out[i(128-part), 2] =
    lhsT(lh_dep[:, chunk]) @ wc_dep, accumulated over m-tiles; ap=2 so
    they are nearly free on the PE.
  - out store in bf16 (host upcasts to f32): halves the 16.8MB/core
    output DMA.  rel-err budget 2e-2 >> bf16 rounding ~2e-3.
  - PE-stall-aware emission: dependent PE groups are emitted >=1 mlp
    group after their producers; dummy warmup matmuls keep the PE busy
    (and its p-state ramped) while batch-0 hidden streams in.
"""

import os
import sys

if "/opt/trn_rl_repo" not in sys.path:
    sys.path.insert(0, "/opt/trn_rl_repo")

import numpy as np

B, L, D = 16, 1024, 1024
MLP = 500
MLP_PAD = 512
NEG_SLOPE = 0.01
N_CORES = 8
B_PER_CORE = B // N_CORES
P = 128
N_MT = MLP_PAD // P  # 4 m-tiles of 128
N_KO = D // P        # 8 d-slices of 128
N_TSUB = L // P      # 8 token subtiles per batch

WARMUP = int(os.environ.get("BB_WARMUP", "14"))

_CACHE = {}


def _build_nc():
    import concourse.tile as tile
    from concourse import bacc, mybir
    from concourse.bass import ts
    from contextlib import ExitStack

    f32 = mybir.dt.float32
    f32r = mybir.dt.float32r
    bf16 = mybir.dt.bfloat16
    Lrelu = mybir.ActivationFunctionType.Lrelu
    Identity = mybir.ActivationFunctionType.Identity
    Add = mybir.AluOpType.add

    nc = bacc.Bacc()

    hid_d = nc.dram_tensor("hidden_t", [B_PER_CORE, D, L], bf16, kind="ExternalInput")
    w_dep_d = nc.dram_tensor("w_dep", [D, MLP_PAD], bf16, kind="ExternalInput")
    w_head_d = nc.dram_tensor("w_head", [D, MLP_PAD], bf16, kind="ExternalInput")
    # f32 consts: cols 0-3 = b_dep per m-tile, 4-7 = b_head, 8-9 = bc
    cf32_d = nc.dram_tensor("consts_f32", [P, 2 * N_MT + 2], f32, kind="ExternalInput")
    wc_dep_d = nc.dram_tensor("wc_dep_t", [P, N_MT, 2], bf16, kind="ExternalInput")
    wc_head_d = nc.dram_tensor("wc_head_t", [P, N_MT, 33], bf16, kind="ExternalInput")
    ones_d = nc.dram_tensor("ones_r", [33, P], f32r, kind="ExternalInput")
    out_d = nc.dram_tensor("out", [B_PER_CORE, L, L, 2], bf16, kind="ExternalOutput")

    with tile.TileContext(nc) as tc:
        with ExitStack() as ctx:
            const = ctx.enter_context(tc.tile_pool(name="const", bufs=1))
            hT_p = ctx.enter_context(tc.tile_pool(name="hT", bufs=2 * N_KO))
            lhh_p = ctx.enter_context(tc.tile_pool(name="lhh", bufs=N_MT))
            lhd_p = ctx.enter_context(tc.tile_pool(name="lhd", bufs=2 * N_MT))
            hs_p = ctx.enter_context(tc.tile_pool(name="hs", bufs=2))
            hbc_p = ctx.enter_context(tc.tile_pool(name="hbc", bufs=4))
            dsb_p = ctx.enter_context(tc.tile_pool(name="dsb", bufs=4))
            out_p = ctx.enter_context(tc.tile_pool(name="outp", bufs=6))
            mlp_ps = ctx.enter_context(tc.tile_pool(name="mlpps", bufs=2, space="PSUM"))
            sc_ps = ctx.enter_context(tc.tile_pool(name="scps", bufs=2, space="PSUM"))

            # ---- constant / weight loads -------------------------------
            # sync: ones (warmup input) first, then batch-0 hidden even kos,
            # then f32 consts + wc tiles.  scalar: w_head chunk for ko0-3,
            # batch-0 hidden odd kos, w_head ko4-7.  gpsimd: w_dep, b1 hidden.
            warm_in = const.tile([1, 512], bf16)
            nc.vector.memset(warm_in, 0.0)
            # trigger both ACT table narrowings during startup idle so no
            # mid-kernel InstLoadActFuncSet lands on the critical path
            warm_act = const.tile([1, 8], f32)
            nc.scalar.activation(warm_act, warm_in[:, 0:8], Lrelu, alpha=NEG_SLOPE)
            nc.scalar.activation(warm_act, warm_in[:, 0:8], Identity)
            w_sb = {}
            w_head_sb = const.tile([P, N_KO, MLP_PAD], bf16)
            w_dep_sb = const.tile([P, N_KO, MLP_PAD], bf16)
            w_sb["dep"], w_sb["head"] = w_dep_sb, w_head_sb

            # hidden tiles: ko-pairs hTp[b][pi] = [P, 2, L] bf16
            hTp = [[hT_p.tile([P, 2, L], bf16, name="hT") for _ in range(N_KO // 2)]
                   for _ in range(B_PER_CORE)]

            def hT(b, ko):
                return hTp[b][ko // 2][:, ko % 2]

            # startup order tuned for batch-0 mt0 pacing: sync carries pairs
            # (0,1),(4,5); scalar interleaves w_head ko-chunks with pairs
            nc.scalar.dma_start(
                w_head_sb[:, 0:2, :],
                w_head_d[0 : 2 * P, :].rearrange("(k p) m -> p k m", p=P),
            )
            nc.sync.dma_start(
                hTp[0][0], hid_d[0, ts(0, 2 * P), :].rearrange("(k p) l -> p k l", p=P)
            )
            nc.scalar.dma_start(
                hTp[0][1], hid_d[0, ts(1, 2 * P), :].rearrange("(k p) l -> p k l", p=P)
            )
            nc.sync.dma_start(
                hTp[0][2], hid_d[0, ts(2, 2 * P), :].rearrange("(k p) l -> p k l", p=P)
            )
            nc.scalar.dma_start(
                w_head_sb[:, 2:4, :],
                w_head_d[2 * P : 4 * P, :].rearrange("(k p) m -> p k m", p=P),
            )
            nc.sync.dma_start(
                hTp[0][3], hid_d[0, ts(3, 2 * P), :].rearrange("(k p) l -> p k l", p=P)
            )
            nc.scalar.dma_start(
                w_head_sb[:, 4:8, :],
                w_head_d[4 * P : 8 * P, :].rearrange("(k p) m -> p k m", p=P),
            )

            ones_sb = const.tile([33, P], f32r)
            nc.sync.dma_start(ones_sb, ones_d[:, :])
            ones_row = {0: ones_sb[0:1, :], 1: ones_sb[32:33, :]}
            cf32_sb = const.tile([P, 2 * N_MT + 2], f32)
            nc.sync.dma_start(cf32_sb, cf32_d[:, :])
            b_sb = {"dep": cf32_sb[:, 0:N_MT], "head": cf32_sb[:, N_MT : 2 * N_MT]}
            bc_sb = cf32_sb[:, 2 * N_MT : 2 * N_MT + 2]
            wc_dep_sb = const.tile([P, N_MT, 2], bf16)
            nc.sync.dma_start(wc_dep_sb, wc_dep_d[:, :, :])
            wc_head_sb = const.tile([P, N_MT, 33], bf16)
            nc.sync.dma_start(wc_head_sb, wc_head_d[:, :, :])

            nc.gpsimd.dma_start(
                w_dep_sb,
                w_dep_d[:, :].rearrange("(k p) m -> p k m", p=P),
            )
            for pi in range(4):
                nc.gpsimd.dma_start(
                    hTp[1][pi],
                    hid_d[1, ts(pi, 2 * P), :].rearrange("(k p) l -> p k l", p=P),
                )

            # ---- emission helpers --------------------------------------
            def emit_dummy(n, ap=512):
                # keep the PE busy/p-state-warm; bf16 => 1 cycle/row
                for _ in range(n):
                    wps = sc_ps.tile([P, ap], f32, name="sc", padded_shape=[P, 1024])
                    nc.tensor.matmul(wps, lhsT=warm_in[:, 0:P], rhs=warm_in[:, 0:ap],
                                     start=True, stop=True)

            lh_head = {}   # (b, mt) -> [P, L] bf16
            lh_dep = {}    # (b, half, mt) -> [P, 512] bf16

            def emit_head_mlp(b, mt, pace=False):
                ps = mlp_ps.tile([P, 1024], f32, name="mlp")
                for ko in range(N_KO):
                    for half in range(2):
                        nc.tensor.matmul(
                            ps[:, ts(half, 512)],
                            lhsT=w_sb["head"][:, ko, ts(mt, P)],
                            rhs=hT(b, ko)[:, ts(half, 512)],
                            start=(ko == 0),
                            stop=(ko == N_KO - 1),
                        )
                    if pace and ko < N_KO - 1:
                        emit_dummy(1, ap=256)
                lh = lhh_p.tile([P, L], bf16, name="lh")
                nc.scalar.activation(lh, ps, Lrelu, bias=b_sb["head"][:, mt : mt + 1],
                                     alpha=NEG_SLOPE)
                lh_head[b, mt] = lh

            def emit_dep_mlp(b, quarter, mt):
                ps = mlp_ps.tile([P, 256], f32, name="mlp", padded_shape=[P, 1024])
                for ko in range(N_KO):
                    nc.tensor.matmul(
                        ps,
                        lhsT=w_sb["dep"][:, ko, ts(mt, P)],
                        rhs=hT(b, ko)[:, ts(quarter, 256)],
                        start=(ko == 0),
                        stop=(ko == N_KO - 1),
                    )
                lh = lhd_p.tile([P, 256], bf16, name="lhd")
                nc.scalar.activation(lh, ps, Lrelu, bias=b_sb["dep"][:, mt : mt + 1],
                                     alpha=NEG_SLOPE)
                lh_dep[b, quarter, mt] = lh

            hs_ps_t = {}
            hs_sb_t = {}

            def emit_hs(b, mt):
                # head scores [2, L]: accumulate over m-tiles, per 512-half
                if mt == 0:
                    hs_ps_t[b] = sc_ps.tile([33, L], f32, name="sc",
                                            padded_shape=[P, 1024])
                for half in range(2):
                    nc.tensor.matmul(
                        hs_ps_t[b][:, ts(half, 512)],
                        lhsT=wc_head_sb[:, mt, :],
                        rhs=lh_head[b, mt][:, ts(half, 512)],
                        start=(mt == 0),
                        stop=(mt == N_MT - 1),
                    )
                if mt == N_MT - 1:
                    hs = hs_p.tile([33, L], f32r, name="hs_sb")
                    nc.vector.tensor_copy(hs, hs_ps_t[b])
                    hs_sb_t[b] = hs

            head_bc = {}

            def emit_bc(b, c):
                ps = sc_ps.tile([P, 1024], f32, name="sc")
                for half in range(2):
                    nc.tensor.matmul(
                        ps[:, ts(half, 512)],
                        lhsT=ones_row[c],
                        rhs=hs_sb_t[b][32 * c : 32 * c + 1, ts(half, 512)],
                        start=True,
                        stop=True,
                    )
                hb = hbc_p.tile([P, L], f32, name="hb")
                nc.vector.tensor_scalar(hb, ps, bc_sb[:, c : c + 1], None, Add)
                head_bc[b, c] = hb

            def emit_tiny_and_out(b, quarter, last=False):
                # dep scores for this quarter: [P(tok), 2] per i-tile
                tiny = sc_ps.tile([P, 2 * 2], f32, name="sc",
                                  padded_shape=[P, 1024])
                for qq in range(2):
                    for mt in range(N_MT):
                        nc.tensor.matmul(
                            tiny[:, 2 * qq : 2 * qq + 2],
                            lhsT=lh_dep[b, quarter, mt][:, ts(qq, P)],
                            rhs=wc_dep_sb[:, mt, :],
                            start=(mt == 0),
                            stop=(mt == N_MT - 1),
                        )
                dsb = dsb_p.tile([P, 2 * 2], f32, name="dsb")
                nc.vector.tensor_copy(dsb, tiny)
                # pairwise add + store.  ACT stays mostly free for lrelu
                # evacuations; paired DMAs on SP/Pool, last quarter split
                # across SP + scalar for the shortest tail.
                def op(sel, dst, srch, dap):
                    if sel == 0:
                        nc.vector.tensor_scalar(dst, srch, dap, None, Add)
                    elif sel == 1:
                        nc.scalar.activation(dst, srch, Identity, bias=dap)
                    else:
                        nc.gpsimd.tensor_scalar(dst, srch, dap, None, Add)

                last_batch = b == B_PER_CORE - 1
                if last_batch and quarter >= 2:
                    # endgame: singles only, transfers spread across queues.
                    # ACT gets no out-op until after its last lrelu; Q2 ops on
                    # DVE+Pool finish before Q3's chain claims DVE.
                    if quarter == 2:
                        units = [((0, 2), nc.sync), ((0, 2), nc.gpsimd)]
                    else:
                        units = [((0, 1), nc.scalar), ((0, 0), nc.sync)]
                    for s, (pk, eng) in enumerate(units):
                        tsub = 2 * quarter + s
                        ot = out_p.tile([P, L, 2], bf16, name="otl")
                        d0 = dsb[:, 2 * s : 2 * s + 1]
                        d1 = dsb[:, 2 * s + 1 : 2 * s + 2]
                        op(pk[0], ot[:, :, 0], head_bc[b, 0], d0)
                        op(pk[1], ot[:, :, 1], head_bc[b, 1], d1)
                        eng.dma_start(out_d[b, ts(tsub, P)], ot)
                    return
                picks = [(0, 2), (1, 0)] if quarter % 2 == 0 else [(0, 1), (2, 0)]
                ot = out_p.tile([P, 2, L, 2], bf16, name="ot")
                for s in range(2):
                    d0 = dsb[:, 2 * s : 2 * s + 1]
                    d1 = dsb[:, 2 * s + 1 : 2 * s + 2]
                    op(picks[s][0], ot[:, s, :, 0], head_bc[b, 0], d0)
                    op(picks[s][1], ot[:, s, :, 1], head_bc[b, 1], d1)
                eng = nc.sync if quarter % 2 == 0 else nc.gpsimd
                eng.dma_start(
                    out_d[b, ts(quarter, 2 * P)].rearrange(
                        "(s p) j c -> p s j c", p=P
                    ),
                    ot,
                )

            # ---- schedule ----------------------------------------------
            # Interleaving keeps every dependent PE group >=1 mlp group
            # behind its producer so the PE never stalls.
            for b in range(B_PER_CORE):
                if b == 0:
                    emit_dummy(WARMUP)
                    emit_head_mlp(b, 0, pace=True)
                    emit_head_mlp(b, 1)
                # for b>0, head mlp 0/1 were emitted inside batch b-1
                if b == 0:
                    # mt0/mt1 are DMA-paced at startup: keep hs well behind
                    emit_head_mlp(b, 2)
                    emit_hs(b, 0)
                    emit_head_mlp(b, 3)
                    emit_hs(b, 1)
                    emit_dep_mlp(b, 0, 0)
                    emit_hs(b, 2)
                else:
                    emit_hs(b, 0)
                    emit_head_mlp(b, 2)
                    emit_hs(b, 1)
                    emit_head_mlp(b, 3)
                    emit_hs(b, 2)
                    emit_dep_mlp(b, 0, 0)
                emit_dep_mlp(b, 0, 1)
                emit_dep_mlp(b, 0, 2)
                emit_dep_mlp(b, 0, 3)
                emit_hs(b, 3)
                emit_dep_mlp(b, 1, 0)
                emit_dep_mlp(b, 1, 1)
                emit_bc(b, 0)
                emit_dep_mlp(b, 1, 2)
                emit_dep_mlp(b, 1, 3)
                emit_bc(b, 1)
                emit_tiny_and_out(b, 0)
                emit_dep_mlp(b, 2, 0)
                emit_dep_mlp(b, 2, 1)
                emit_dep_mlp(b, 2, 2)
                emit_dep_mlp(b, 2, 3)
                emit_tiny_and_out(b, 1)
                emit_dep_mlp(b, 3, 0)
                emit_dep_mlp(b, 3, 1)
                if b + 1 < B_PER_CORE:
                    emit_dep_mlp(b, 3, 2)
                    emit_dep_mlp(b, 3, 3)
                    emit_tiny_and_out(b, 2)
                    emit_head_mlp(b + 1, 0)
                    emit_tiny_and_out(b, 3)
                    emit_head_mlp(b + 1, 1)
                else:
                    # last batch: drain quarter 2 before Q3's mlp finishes so
                    # only Q3's chain remains in the tail
                    emit_tiny_and_out(b, 2)
                    emit_dep_mlp(b, 3, 2)
                    emit_dep_mlp(b, 3, 3)
                    emit_tiny_and_out(b, 3, last=True)

    nc.compile()
    return nc


def _prep_consts(W_dep, b_dep, W_head, b_head, Wc, bc):
    import ml_dtypes

    f = np.float32
    bf = ml_dtypes.bfloat16

    def pad_w(W):
        Wp = np.zeros((D, MLP_PAD), f)
        Wp[:, :MLP] = W
        return Wp.astype(bf)

    def bias_t(bvec):
        bp = np.zeros((MLP_PAD,), f)
        bp[:MLP] = bvec
        return bp.reshape(N_MT, P).T  # [P, N_MT]

    cf32 = np.empty((P, 2 * N_MT + 2), f)
    cf32[:, 0:N_MT] = bias_t(b_dep)
    cf32[:, N_MT : 2 * N_MT] = bias_t(b_head)
    cf32[:, 2 * N_MT :] = np.broadcast_to(bc.astype(f), (P, 2))

    def wc_t(wc_half, width=2, stride=1):
        wcp = np.zeros((MLP_PAD, 2), f)
        wcp[:MLP] = wc_half
        wct = wcp.reshape(N_MT, P, 2).transpose(1, 0, 2)  # [P, N_MT, 2]
        out = np.zeros((P, N_MT, width), f)
        out[:, :, 0] = wct[:, :, 0]
        out[:, :, stride] = wct[:, :, 1]
        return out.astype(bf).copy()

    return {
        "w_dep": pad_w(W_dep),
        "w_head": pad_w(W_head),
        "consts_f32": cf32,
        "wc_dep_t": wc_t(Wc[:MLP]),
        "wc_head_t": wc_t(Wc[MLP:], width=33, stride=32),
        "ones_r": np.ones((33, P), f),
    }


def kernel(hidden_state, W_dep, b_dep, W_head, b_head, Wc, bc):
    import ml_dtypes
    from concourse.bass_utils import run_bass_kernel_spmd

    bf = ml_dtypes.bfloat16
    hidden_state = np.asarray(hidden_state, dtype=np.float32)
    consts = _prep_consts(
        np.asarray(W_dep, np.float32),
        np.asarray(b_dep, np.float32),
        np.asarray(W_head, np.float32),
        np.asarray(b_head, np.float32),
        np.asarray(Wc, np.float32),
        np.asarray(bc, np.float32),
    )

    if "nc" not in _CACHE:
        _CACHE["nc"] = _build_nc()
    nc = _CACHE["nc"]

    hbf = hidden_state.astype(bf)
    in_maps = []
    for k in range(N_CORES):
        sl = hbf[k * B_PER_CORE : (k + 1) * B_PER_CORE]
        m = {"hidden_t": np.ascontiguousarray(sl.transpose(0, 2, 1))}
        m.update(consts)
        in_maps.append(m)

    trace = bool(int(os.environ.get("BB_TRACE", "0")))
    if not trace:
        # The NTFF profiling hook (antenv.axon_hooks) is absent in this
        # container; a stray BASS_TRACE=1 would crash the run. Force off.
        os.environ["BASS_NEVER_TRACE"] = "1"
    res = run_bass_kernel_spmd(nc, in_maps, list(range(N_CORES)), trace=trace)
    _CACHE["last_results"] = res
    out = np.concatenate(
        [np.asarray(res.results[k]["out"], dtype=np.float32) for k in range(N_CORES)],
        axis=0,
    )
    return out


# revision 21
# speedup vs baseline: 1.0472x; 1.0252x over previous
"""Trainium2 Bass kernel for nn_BinaryBiaffine2 (biaffine dependency scorer).

Math (per batch b):
    h_dep  = leaky_relu(hidden @ W_dep  + b_dep)             [L, 500]
    h_head = leaky_relu(hidden @ W_head + b_head)            [L, 500]
    dep    = h_dep  @ Wc[:500]                               [L, 2]
    head   = h_head @ Wc[500:]                               [L, 2]
    out[i, j, c] = dep[i, c] + head[j, c] + bc[c]            [L, L, 2]

Sharding: data-parallel over batch, 2 batches per core on 8 cores.

v3 strategy (vs v2's 108.5us):
  - hidden is transposed to [D, L] on the HOST and fed as bf16, so the
    kernel streams hT tiles [d=128, tok] straight from DRAM: no PE
    transposes, no PSUM round-trip, no DVE copies for them.
  - weights bf16 (1 cycle/row on PE, half the DMA bytes).
  - leaky_relu fused into ONE scalar-engine activation (Lrelu, alpha).
  - head scores [2, L] via M=2 matmuls; partition-broadcast via a
    ones-row matmul (f32r); +bc folded into the PSUM->SBUF copy.
  - dep scores via per-i-tile tiny matmuls: out[i(128-part), 2] =
    lhsT(lh_dep[:, chunk]) @ wc_dep, accumulated over m-tiles; ap=2 so
    they are nearly free on the PE.
  - out store in bf16 (host upcasts to f32): halves the 16.8MB/core
    output DMA.  rel-err budget 2e-2 >> bf16 rounding ~2e-3.
  - PE-stall-aware emission: dependent PE groups are emitted >=1 mlp
    group after their producers; dummy warmup matmuls keep the PE busy
    (and its p-state ramped) while batch-0 hidden streams in.
"""

import os
import sys

if "/opt/trn_rl_repo" not in sys.path:
    sys.path.insert(0, "/opt/trn_rl_repo")

import numpy as np

B, L, D = 16, 1024, 1024
MLP = 500
MLP_PAD = 512
NEG_SLOPE = 0.01
N_CORES = 8
B_PER_CORE = B // N_CORES
P = 128
N_MT = MLP_PAD // P  # 4 m-tiles of 128
N_KO = D // P        # 8 d-slices of 128
N_TSUB = L // P      # 8 token subtiles per batch

WARMUP = int(os.environ.get("BB_WARMUP", "10"))
PACE = int(os.environ.get("BB_PACE", "1"))

_CACHE = {}


def _build_nc():
    import concourse.tile as tile
    from concourse import bacc, mybir
    from concourse.bass import ts
    from contextlib import ExitStack

    f32 = mybir.dt.float32
    f32r = mybir.dt.float32r
    bf16 = mybir.dt.bfloat16
    Lrelu = mybir.ActivationFunctionType.Lrelu
    Identity = mybir.ActivationFunctionType.Identity
    Add = mybir.AluOpType.add

    nc = bacc.Bacc()

    hid_d = nc.dram_tensor("hidden_t", [B_PER_CORE, D, L], bf16, kind="ExternalInput")
    w_dep_d = nc.dram_tensor("w_dep", [D, MLP_PAD], bf16, kind="ExternalInput")
    w_head_d = nc.dram_tensor("w_head", [D, MLP_PAD], bf16, kind="ExternalInput")
    # f32 consts: cols 0-3 = b_dep per m-tile, 4-7 = b_head, 8-9 = bc
    cf32_d = nc.dram_tensor("consts_f32", [P, 2 * N_MT + 6], f32, kind="ExternalInput")
    wc_dep_d = nc.dram_tensor("wc_dep_t", [P, N_MT, 2], bf16, kind="ExternalInput")
    wc_head_d = nc.dram_tensor("wc_head_t", [P, N_MT, 33], bf16, kind="ExternalInput")
    out_d = nc.dram_tensor("out", [B_PER_CORE, L, L, 2], bf16, kind="ExternalOutput")

    with tile.TileContext(nc) as tc:
        with ExitStack() as ctx:
            const = ctx.enter_context(tc.tile_pool(name="const", bufs=1))
            hT_p = ctx.enter_context(tc.tile_pool(name="hT", bufs=2 * N_KO))
            lhh_p = ctx.enter_context(tc.tile_pool(name="lhh", bufs=N_MT))
            lhd_p = ctx.enter_context(tc.tile_pool(name="lhd", bufs=2 * N_MT))
            hs_p = ctx.enter_context(tc.tile_pool(name="hs", bufs=2))
            hbc_p = ctx.enter_context(tc.tile_pool(name="hbc", bufs=4))
            dsb_p = ctx.enter_context(tc.tile_pool(name="dsb", bufs=4))
            out_p = ctx.enter_context(tc.tile_pool(name="outp", bufs=6))
            mlp_ps = ctx.enter_context(tc.tile_pool(name="mlpps", bufs=2, space="PSUM"))
            sc_ps = ctx.enter_context(tc.tile_pool(name="scps", bufs=2, space="PSUM"))

            # ---- constant / weight loads -------------------------------
            # sync: ones (warmup input) first, then batch-0 hidden even kos,
            # then f32 consts + wc tiles.  scalar: w_head chunk for ko0-3,
            # batch-0 hidden odd kos, w_head ko4-7.  gpsimd: w_dep, b1 hidden.
            warm_in = const.tile([1, 512], bf16)
            nc.vector.memset(warm_in, 0.0)
            # trigger both ACT table narrowings during startup idle so no
            # mid-kernel InstLoadActFuncSet lands on the critical path
            warm_act = const.tile([1, 8], f32)
            nc.scalar.activation(warm_act, warm_in[:, 0:8], Lrelu, alpha=NEG_SLOPE)
            nc.scalar.activation(warm_act, warm_in[:, 0:8], Identity)
            w_sb = {}
            w_head_sb = const.tile([P, N_KO, MLP_PAD], bf16)
            w_dep_sb = const.tile([P, N_KO, MLP_PAD], bf16)
            w_sb["dep"], w_sb["head"] = w_dep_sb, w_head_sb

            # hidden tiles: ko-pairs hTp[b][pi] = [P, 2, L] bf16
            hTp = [[hT_p.tile([P, 2, L], bf16, name="hT") for _ in range(N_KO // 2)]
                   for _ in range(B_PER_CORE)]

            def hT(b, ko):
                return hTp[b][ko // 2][:, ko % 2]

            # startup order tuned for batch-0 mt0 pacing: sync carries pairs
            # (0,1),(4,5); scalar interleaves w_head ko-chunks with pairs
            nc.scalar.dma_start(
                w_head_sb[:, 0:2, :],
                w_head_d[0 : 2 * P, :].rearrange("(k p) m -> p k m", p=P),
            )
            nc.sync.dma_start(
                hTp[0][0], hid_d[0, ts(0, 2 * P), :].rearrange("(k p) l -> p k l", p=P)
            )
            nc.scalar.dma_start(
                hTp[0][1], hid_d[0, ts(1, 2 * P), :].rearrange("(k p) l -> p k l", p=P)
            )
            nc.sync.dma_start(
                hTp[0][2], hid_d[0, ts(2, 2 * P), :].rearrange("(k p) l -> p k l", p=P)
            )
            nc.scalar.dma_start(
                w_head_sb[:, 2:4, :],
                w_head_d[2 * P : 4 * P, :].rearrange("(k p) m -> p k m", p=P),
            )
            nc.sync.dma_start(
                hTp[0][3], hid_d[0, ts(3, 2 * P), :].rearrange("(k p) l -> p k l", p=P)
            )
            nc.scalar.dma_start(
                w_head_sb[:, 4:8, :],
                w_head_d[4 * P : 8 * P, :].rearrange("(k p) m -> p k m", p=P),
            )

            cf32_sb = const.tile([P, 2 * N_MT + 6], f32)
            nc.sync.dma_start(cf32_sb, cf32_d[:, :])
            b_sb = {"dep": cf32_sb[:, 0:N_MT], "head": cf32_sb[:, N_MT : 2 * N_MT]}
            bc_sb = cf32_sb[:, 2 * N_MT : 2 * N_MT + 2]
            bc_pat = cf32_sb[:, 2 * N_MT + 2 : 2 * N_MT + 6]
            wc_dep_sb = const.tile([P, N_MT, 2], bf16)
            nc.sync.dma_start(wc_dep_sb, wc_dep_d[:, :, :])
            wc_head_sb = const.tile([P, N_MT, 33], bf16)
            nc.sync.dma_start(wc_head_sb, wc_head_d[:, :, :])

            nc.gpsimd.dma_start(
                w_dep_sb,
                w_dep_d[:, :].rearrange("(k p) m -> p k m", p=P),
            )
            for pi in range(4):
                nc.gpsimd.dma_start(
                    hTp[1][pi],
                    hid_d[1, ts(pi, 2 * P), :].rearrange("(k p) l -> p k l", p=P),
                )

            # ---- emission helpers --------------------------------------
            def emit_dummy(n, ap=512):
                # keep the PE busy/p-state-warm; bf16 => 1 cycle/row
                for _ in range(n):
                    wps = sc_ps.tile([P, ap], f32, name="sc", padded_shape=[P, 1024])
                    nc.tensor.matmul(wps, lhsT=warm_in[:, 0:P], rhs=warm_in[:, 0:ap],
                                     start=True, stop=True)

            lh_head = {}   # (b, mt) -> [P, L] bf16
            lh_dep = {}    # (b, half, mt) -> [P, 512] bf16

            def emit_head_mlp(b, mt, pace=False):
                ps = mlp_ps.tile([P, 1024], f32, name="mlp")
                for ko in range(N_KO):
                    for half in range(2):
                        nc.tensor.matmul(
                            ps[:, ts(half, 512)],
                            lhsT=w_sb["head"][:, ko, ts(mt, P)],
                            rhs=hT(b, ko)[:, ts(half, 512)],
                            start=(ko == 0),
                            stop=(ko == N_KO - 1),
                        )
                    if pace and PACE and ko < N_KO - 1:
                        emit_dummy(PACE, ap=256)
                lh = lhh_p.tile([P, L], bf16, name="lh")
                nc.scalar.activation(lh, ps, Lrelu, bias=b_sb["head"][:, mt : mt + 1],
                                     alpha=NEG_SLOPE)
                lh_head[b, mt] = lh

            def emit_dep_mlp(b, quarter, mt):
                ps = mlp_ps.tile([P, 256], f32, name="mlp", padded_shape=[P, 1024])
                for ko in range(N_KO):
                    nc.tensor.matmul(
                        ps,
                        lhsT=w_sb["dep"][:, ko, ts(mt, P)],
                        rhs=hT(b, ko)[:, ts(quarter, 256)],
                        start=(ko == 0),
                        stop=(ko == N_KO - 1),
                    )
                lh = lhd_p.tile([P, 256], bf16, name="lhd")
                nc.scalar.activation(lh, ps, Lrelu, bias=b_sb["dep"][:, mt : mt + 1],
                                     alpha=NEG_SLOPE)
                lh_dep[b, quarter, mt] = lh

            hs_ps_t = {}
            hs_sb_t = {}
            hs1_sb_t = {}

            def emit_hs(b, mt):
                # head scores [2, L]: accumulate over m-tiles, per 512-half
                if mt == 0:
                    hs_ps_t[b] = sc_ps.tile([33, L], f32, name="sc",
                                            padded_shape=[P, 1024])
                for half in range(2):
                    nc.tensor.matmul(
                        hs_ps_t[b][:, ts(half, 512)],
                        lhsT=wc_head_sb[:, mt, :],
                        rhs=lh_head[b, mt][:, ts(half, 512)],
                        start=(mt == 0),
                        stop=(mt == N_MT - 1),
                    )
                if mt == N_MT - 1:
                    hs = hs_p.tile([33, L], f32, name="hs_sb")
                    nc.vector.tensor_copy(hs, hs_ps_t[b])
                    hs_sb_t[b] = hs
                    # channel 1 lives at partition 32; partition_broadcast
                    # only reads partition 0, so DMA its row to a base-0 tile
                    hs1 = hs_p.tile([1, L], f32, name="hs1_sb")
                    nc.sync.dma_start(hs1, hs[32:33, :])
                    hs1_sb_t[b] = hs1

            head_bc = {}

            def emit_bc(b, c):
                # partition-broadcast head scores row c across 128 partitions
                # (Pool SWDGE); +bc is folded into the dep-score tile instead
                hb = hbc_p.tile([P, L], f32, name="hb")
                src_row = hs_sb_t[b][0:1, :] if c == 0 else hs1_sb_t[b][0:1, :]
                nc.gpsimd.partition_broadcast(hb, src_row)
                head_bc[b, c] = hb

            def emit_tiny_and_out(b, quarter, last=False):
                # dep scores for this quarter: [P(tok), 2] per i-tile
                tiny = sc_ps.tile([P, 2 * 2], f32, name="sc",
                                  padded_shape=[P, 1024])
                for qq in range(2):
                    for mt in range(N_MT):
                        nc.tensor.matmul(
                            tiny[:, 2 * qq : 2 * qq + 2],
                            lhsT=lh_dep[b, quarter, mt][:, ts(qq, P)],
                            rhs=wc_dep_sb[:, mt, :],
                            start=(mt == 0),
                            stop=(mt == N_MT - 1),
                        )
                dsb = dsb_p.tile([P, 2 * 2], f32, name="dsb")
                nc.vector.tensor_add(dsb, tiny, bc_pat)
                # pairwise add + store.  ACT stays mostly free for lrelu
                # evacuations; paired DMAs on SP/Pool, last quarter split
                # across SP + scalar for the shortest tail.
                def op(sel, dst, srch, dap):
                    if sel == 0:
                        nc.vector.tensor_scalar(dst, srch, dap, None, Add)
                    elif sel == 1:
                        nc.scalar.activation(dst, srch, Identity, bias=dap)
                    else:
                        nc.gpsimd.tensor_scalar(dst, srch, dap, None, Add)

                last_batch = b == B_PER_CORE - 1
                if last_batch and quarter >= 2:
                    # endgame: singles only, transfers spread across queues.
                    # ACT gets no out-op until after its last lrelu; Q2 ops on
                    # DVE+Pool finish before Q3's chain claims DVE.
                    if quarter == 2:
                        units = [((0, 2), nc.sync), ((0, 2), nc.gpsimd)]
                    else:
                        units = [((0, 1), nc.scalar), ((0, 0), nc.sync)]
                    for s, (pk, eng) in enumerate(units):
                        tsub = 2 * quarter + s
                        ot = out_p.tile([P, L, 2], bf16, name="otl")
                        d0 = dsb[:, 2 * s : 2 * s + 1]
                        d1 = dsb[:, 2 * s + 1 : 2 * s + 2]
                        op(pk[0], ot[:, :, 0], head_bc[b, 0], d0)
                        op(pk[1], ot[:, :, 1], head_bc[b, 1], d1)
                        eng.dma_start(out_d[b, ts(tsub, P)], ot)
                    return
                picks = [(0, 2), (0, 0)] if quarter % 2 == 0 else [(0, 0), (2, 0)]
                ot = out_p.tile([P, 2, L, 2], bf16, name="ot")
                for s in range(2):
                    d0 = dsb[:, 2 * s : 2 * s + 1]
                    d1 = dsb[:, 2 * s + 1 : 2 * s + 2]
                    op(picks[s][0], ot[:, s, :, 0], head_bc[b, 0], d0)
                    op(picks[s][1], ot[:, s, :, 1], head_bc[b, 1], d1)
                eng = nc.sync if quarter % 2 == 0 else nc.gpsimd
                eng.dma_start(
                    out_d[b, ts(quarter, 2 * P)].rearrange(
                        "(s p) j c -> p s j c", p=P
                    ),
                    ot,
                )

            # ---- schedule ----------------------------------------------
            # Interleaving keeps every dependent PE group >=1 mlp group
            # behind its producer so the PE never stalls.
            for b in range(B_PER_CORE):
                if b == 0:
                    emit_dummy(WARMUP)
                    emit_head_mlp(b, 0, pace=True)
                    emit_head_mlp(b, 1)
                # for b>0, head mlp 0/1 were emitted inside batch b-1
                if b == 0:
                    # mt0/mt1 are DMA-paced at startup: keep hs well behind
                    emit_head_mlp(b, 2)
                    emit_hs(b, 0)
                    emit_head_mlp(b, 3)
                    emit_hs(b, 1)
                    emit_dep_mlp(b, 0, 0)
                    emit_hs(b, 2)
                else:
                    emit_hs(b, 0)
                    emit_head_mlp(b, 2)
                    emit_hs(b, 1)
                    emit_head_mlp(b, 3)
                    emit_hs(b, 2)
                    emit_dep_mlp(b, 0, 0)
                emit_dep_mlp(b, 0, 1)
                emit_dep_mlp(b, 0, 2)
                emit_dep_mlp(b, 0, 3)
                emit_hs(b, 3)
                emit_dep_mlp(b, 1, 0)
                emit_dep_mlp(b, 1, 1)
                emit_bc(b, 0)
                emit_dep_mlp(b, 1, 2)
                emit_dep_mlp(b, 1, 3)
                emit_bc(b, 1)
                emit_tiny_and_out(b, 0)
                emit_dep_mlp(b, 2, 0)
                emit_dep_mlp(b, 2, 1)
                emit_dep_mlp(b, 2, 2)
                emit_dep_mlp(b, 2, 3)
                emit_tiny_and_out(b, 1)
                emit_dep_mlp(b, 3, 0)
                emit_dep_mlp(b, 3, 1)
                if b + 1 < B_PER_CORE:
                    emit_dep_mlp(b, 3, 2)
                    emit_dep_mlp(b, 3, 3)
                    emit_tiny_and_out(b, 2)
                    emit_head_mlp(b + 1, 0)
                    emit_tiny_and_out(b, 3)
                    emit_head_mlp(b + 1, 1)
                else:
                    # last batch: drain quarter 2 before Q3's mlp finishes so
                    # only Q3's chain remains in the tail
                    emit_tiny_and_out(b, 2)
                    emit_dep_mlp(b, 3, 2)
                    emit_dep_mlp(b, 3, 3)
                    emit_tiny_and_out(b, 3, last=True)

    nc.compile()
    return nc


def _prep_consts(W_dep, b_dep, W_head, b_head, Wc, bc):
    import ml_dtypes

    f = np.float32
    bf = ml_dtypes.bfloat16

    def pad_w(W):
        Wp = np.zeros((D, MLP_PAD), f)
        Wp[:, :MLP] = W
        return Wp.astype(bf)

    def bias_t(bvec):
        bp = np.zeros((MLP_PAD,), f)
        bp[:MLP] = bvec
        return bp.reshape(N_MT, P).T  # [P, N_MT]

    cf32 = np.empty((P, 2 * N_MT + 6), f)
    cf32[:, 0:N_MT] = bias_t(b_dep)
    cf32[:, N_MT : 2 * N_MT] = bias_t(b_head)
    cf32[:, 2 * N_MT : 2 * N_MT + 2] = np.broadcast_to(bc.astype(f), (P, 2))
    cf32[:, 2 * N_MT + 2 :] = np.broadcast_to(
        np.tile(bc.astype(f), 2), (P, 4)
    )

    def wc_t(wc_half, width=2, stride=1):
        wcp = np.zeros((MLP_PAD, 2), f)
        wcp[:MLP] = wc_half
        wct = wcp.reshape(N_MT, P, 2).transpose(1, 0, 2)  # [P, N_MT, 2]
        out = np.zeros((P, N_MT, width), f)
        out[:, :, 0] = wct[:, :, 0]
        out[:, :, stride] = wct[:, :, 1]
        return out.astype(bf).copy()

    return {
        "w_dep": pad_w(W_dep),
        "w_head": pad_w(W_head),
        "consts_f32": cf32,
        "wc_dep_t": wc_t(Wc[:MLP]),
        "wc_head_t": wc_t(Wc[MLP:], width=33, stride=32),
    }


def kernel(hidden_state, W_dep, b_dep, W_head, b_head, Wc, bc):
    import ml_dtypes
    from concourse.bass_utils import run_bass_kernel_spmd

    bf = ml_dtypes.bfloat16
    hidden_state = np.asarray(hidden_state, dtype=np.float32)
    consts = _prep_consts(
        np.asarray(W_dep, np.float32),
        np.asarray(b_dep, np.float32),
        np.asarray(W_head, np.float32),
        np.asarray(b_head, np.float32),
        np.asarray(Wc, np.float32),
        np.asarray(bc, np.float32),
    )

    if "nc" not in _CACHE:
        _CACHE["nc"] = _build_nc()
    nc = _CACHE["nc"]

    hbf = hidden_state.astype(bf)
    in_maps = []
    for k in range(N_CORES):
        sl = hbf[k * B_PER_CORE : (k + 1) * B_PER_CORE]
        m = {"hidden_t": np.ascontiguousarray(sl.transpose(0, 2, 1))}
        m.update(consts)
        in_maps.append(m)

    trace = bool(int(os.environ.get("BB_TRACE", "0")))
    if not trace:
        # The NTFF profiling hook (antenv.axon_hooks) is absent in this
        # container; a stray BASS_TRACE=1 would crash the run. Force off.
        os.environ["BASS_NEVER_TRACE"] = "1"
    res = run_bass_kernel_spmd(nc, in_maps, list(range(N_CORES)), trace=trace)
    _CACHE["last_results"] = res
    out = np.concatenate(
        [np.asarray(res.results[k]["out"], dtype=np.float32) for k in range(N_CORES)],
        axis=0,
    )
    return out
